# revision 55
# baseline (speedup 1.0000x reference)
"""Trainium2 Bass kernel for nn_LocalAttention (B=4, S=1024, E=768, H=12, windows 16/64/256).

Math notes (exact, not approximate):
  - The reference multiplies scores by each band mask progressively; since
    band(16) is a subset of band(64)/band(256) and the attention mask is
    idempotent (0/1), all three softmax inputs are the identical matrix
    raw * band16 * am.  Hence combined = sum(window_weights) * softmax(masked).
  - The softmax is over the FULL row (out-of-band entries are 0, contributing
    exp(0)=1 each).  With E = exp(s_masked) on the 160-wide banded tile:
        Z_q   = sum_tile(E) + (S - 160)
        ctx_q = E_tile @ V_tile + (V_sum - V_tilesum)      per head
    because in-tile out-of-band entries contribute exactly 1*v (E=exp(0)=1)
    and padded halo rows have v=0 on both sides of the identity.
  - Scores are computed TRANSPOSED (k on partitions) so the exp'd tile is
    directly the lhsT of the context matmul -- no PE transposes of E needed.
    A ones-column appended to V yields sum_tile(E) inside the same matmul,
    and a final K=1 matmul adds the (V_sum - V_tilesum | S-160) row, so Z
    and ctx pop out of one PSUM tile per 6 heads.
  - Sharding: core c -> batch c//2, query rows [(c%2)*512, +512).  k/v are
    projected for a zero-padded halo window of 544 rows.  All heads stay on
    one core so LayerNorm needs no communication.
  - All matmul operands are bf16 (fp32r emits 2 MATMUL instructions per op
    and streams at half rate); PSUM accumulation stays fp32.
"""

import os
import sys

sys.path.insert(0, "/opt/trn_rl_repo")



import numpy as np
import ml_dtypes

import concourse.bass as bass  # noqa: F401
import concourse.mybir as mybir
import concourse.tile as tile
from concourse import bacc
from concourse.bass_utils import run_bass_kernel_spmd
from concourse.masks import make_identity

B, S, E, H, D = 4, 1024, 768, 12, 64
N_CORES = 8
R = 512               # query rows per core
HALO = 16             # band half-width that survives the mask product
KW = R + 2 * HALO     # 544-row k/v halo window
NT = R // 128         # 4 query tiles per core
TW = 160              # banded k-width per 128-row query tile
IB = E // 128         # 6 feature blocks
LN_EPS = 1e-5

f32 = mybir.dt.float32
bf16 = mybir.dt.bfloat16
AF = mybir.ActivationFunctionType
ALU = mybir.AluOpType

_cache = {}

bf = ml_dtypes.bfloat16


def _emit(nc, tc, dram, has_bias, unit_gamma):
    sync = nc.sync

    with tc.tile_pool(name="const", bufs=1) as cp, \
         tc.tile_pool(name="work", bufs=2) as wp, \
         tc.tile_pool(name="psA", bufs=2, space="PSUM") as pA, \
         tc.tile_pool(name="psS", bufs=5, space="PSUM") as pS, \
         tc.tile_pool(name="psS1", bufs=1, space="PSUM") as pS1:
        pT = pA  # transposes share the projection psum slots (8-bank budget)

        # ---------------- small constants (scalar-engine DMA queue) ----------
        # NOTE: every matmul in this kernel uses the full K=128 contraction.
        # Sub-128 contractions select row-tiled PE modes whose concurrent
        # drains crash the hardware (probed); zero-padding the contraction is
        # free since matmul cycles scale with N only.
        ident = cp.tile([128, 128], bf16, tag="ident")
        make_identity(nc, ident[:])
        onesr = cp.tile([128, 128], bf16, tag="onesr")
        nc.gpsimd.memset(onesr[:], 0.0)
        nc.gpsimd.memset(onesr[0:1, :], 1.0)
        epst = cp.tile([128, 1], f32, tag="epst")
        nc.vector.memset(epst[:], LN_EPS)

        band0 = cp.tile([128, 512], bf16, tag="band0")
        nc.scalar.dma_start(band0[:], dram["band0"][:])
        band1 = cp.tile([32, 512], bf16, tag="band1")
        nc.scalar.dma_start(band1[:], dram["band1"][:])
        amc0 = cp.tile([128, NT], f32, tag="amc0")
        nc.scalar.dma_start(amc0[:], dram["amcol0"][:])
        amc1 = cp.tile([32, NT], f32, tag="amc1")
        nc.scalar.dma_start(amc1[:], dram["amcol1"][:])
        vdzsb = cp.tile([128, NT * 780], bf16, tag="vdzsb")
        nc.gpsimd.memset(vdzsb[:], 0.0)
        nc.scalar.dma_start(vdzsb[0:1, :], dram["vdz"][:])
        bkc = cp.tile([128, IB], f32, tag="bkc")
        nc.scalar.dma_start(bkc[:], dram["bk_col"][:])
        if has_bias:
            browp = cp.tile([128, 2 * E], bf16, tag="browp")
            nc.gpsimd.memset(browp[:], 0.0)
            nc.scalar.dma_start(browp[0:1, :], dram["brow"][:])
        if not unit_gamma:
            growp = cp.tile([128, 2 * E], bf16, tag="growp")
            nc.gpsimd.memset(growp[:], 0.0)
            nc.scalar.dma_start(growp[0:1, :], dram["grow"][:])

        # ---------------- big inputs (sync-engine DMA queue, in use order) ---
        # qT/Wq halves so the first Q chain starts after ~1MB instead of 2MB
        qTp_in = cp.tile([128, IB * R], bf16, tag="qTp")
        Wq = cp.tile([128, IB * E], bf16, tag="Wq")
        sync.dma_start(qTp_in[:, 0: 3 * R], dram["qT"][:, 0: 3 * R])
        sync.dma_start(Wq[:, 0: 3 * E], dram["WqT"][:, 0: 3 * E])
        sync.dma_start(qTp_in[:, 3 * R:], dram["qT"][:, 3 * R:])
        sync.dma_start(Wq[:, 3 * E:], dram["WqT"][:, 3 * E:])
        kTp_in = cp.tile([128, IB * KW], bf16, tag="kTp")
        Wk = cp.tile([128, IB * E], bf16, tag="Wk")
        sync.dma_start(Wk[:, 0: 3 * E], dram["WkT"][:, 0: 3 * E])
        sync.dma_start(kTp_in[:, 0: 3 * KW], dram["kT"][:, 0: 3 * KW])
        sync.dma_start(Wk[:, 3 * E:], dram["WkT"][:, 3 * E:])
        sync.dma_start(kTp_in[:, 3 * KW:], dram["kT"][:, 3 * KW:])
        xvp_in = cp.tile([128, IB * KW], bf16, tag="xvp")
        Wv = cp.tile([128, IB * E], bf16, tag="Wv")
        sync.dma_start(Wv[:, 0: 3 * E], dram["WvT"][:, 0: 3 * E])
        sync.dma_start(xvp_in[:, 0: 3 * KW], dram["vT"][:, 0: 3 * KW])
        sync.dma_start(Wv[:, 3 * E:], dram["WvT"][:, 3 * E:])
        sync.dma_start(xvp_in[:, 3 * KW:], dram["vT"][:, 3 * KW:])

        # gamma/beta broadcast to [128, E] (PE ones-broadcast), fp32 out
        if not unit_gamma:
            gb = cp.tile([128, E], f32, tag="gb")
            bb = cp.tile([128, E], f32, tag="bb")
            for base, dst in ((0, gb), (E, bb)):
                for c0, w in ((0, 512), (512, 256)):
                    ps = pA.tile([128, 512], f32, tag="A")
                    nc.tensor.matmul(ps[:, :w], onesr[:],
                                     growp[:, base + c0:base + c0 + w],
                                     start=True, stop=True)
                    nc.scalar.copy(dst[:, c0:c0 + w], ps[:, :w])

        # ---------------- stage A: projections (all bf16 operands) ----------
        # Q projection, token-major: q_tok[tt] [128 tok, 768]
        q_tok = []
        for tt in range(NT):
            qt = cp.tile([128, E], bf16, tag=f"qtok{tt}")
            for c0, w in ((0, 512), (512, 256)):
                ps = pA.tile([128, 512], f32, tag="A")
                for ib in range(IB):
                    nc.tensor.matmul(ps[:, :w],
                                     qTp_in[:, ib * R + tt * 128: ib * R + (tt + 1) * 128],
                                     Wq[:, ib * E + c0: ib * E + c0 + w],
                                     start=(ib == 0),
                                     stop=(ib == IB - 1 and not has_bias))
                if has_bias:
                    nc.tensor.matmul(ps[:, :w], onesr[:], browp[:, c0:c0 + w],
                                     start=False, stop=True)
                nc.vector.tensor_copy(qt[:, c0:c0 + w], ps[:, :w])
            q_tok.append(qt)

        # q -> feature-major via PE transpose (bf16 PSUM), per tile.  Two
        # half-masked copies: qTe keeps rows 0:64 (even heads' d-dims) with
        # rows 64:128 zero, qTo the reverse -- so score matmuls contract over
        # the full 128 partitions (the foreign head's terms hit zeros).
        qTe, qTo = [], []
        for tt in range(NT):
            te = cp.tile([128, E], bf16, tag=f"qTe{tt}")
            nc.gpsimd.memset(te[64:128, :], 0.0)
            to = cp.tile([128, E], bf16, tag=f"qTo{tt}")
            nc.gpsimd.memset(to[0:64, :], 0.0)
            qTe.append(te)
            qTo.append(to)
        for tt in range(NT):
            qp = pT.tile([128, E], bf16, tag="A")
            for c in range(IB):
                nc.tensor.transpose(qp[:, c * 128:(c + 1) * 128],
                                    q_tok[tt][:, c * 128:(c + 1) * 128], ident[:])
            nc.vector.tensor_copy(qTe[tt][0:64, :], qp[0:64, :])
            nc.vector.tensor_copy(qTo[tt][64:128, :], qp[64:128, :])

        # K projection, feature-major: kT_sb[ob] [128 e_out, 544 tok] + bk
        kT_sb = []
        for ob in range(IB):
            kt = cp.tile([128, KW], bf16, tag=f"kT{ob}")
            for c0, w in ((0, 512), (512, KW - 512)):
                ps = pA.tile([128, 512], f32, tag="A")
                for ib in range(IB):
                    nc.tensor.matmul(ps[:, :w],
                                     Wk[:, ib * E + ob * 128: ib * E + (ob + 1) * 128],
                                     kTp_in[:, ib * KW + c0: ib * KW + c0 + w],
                                     start=(ib == 0), stop=(ib == IB - 1))
                nc.vector.tensor_scalar_add(kt[:, c0:c0 + w], ps[:, :w],
                                            bkc[:, ob:ob + 1])
            kT_sb.append(kt)

        # V projection, token-major into interleaved [ctx|ones] layout:
        # vplus[t5] [128 tok, 12*65]; col h*65+64 is the ones column.
        # Chains are emitted lazily inside the stage-B loop so the PE does
        # scores work while the V inputs are still streaming in, instead of
        # stalling between the K projection and stage B.
        vplus = []

        def emit_vchain(t5):
            rows = 128 if t5 < 4 else KW - 4 * 128
            vp = cp.tile([128, H * 65], bf16, tag=f"vplus{t5}",
                         name=f"vplus{t5}")
            for c0, w in ((0, 512), (512, 256)):
                ps = pA.tile([128, 512], f32, tag="A", name=f"vps{t5}_{c0}")
                for ib in range(IB):
                    nc.tensor.matmul(ps[:rows, :w],
                                     xvp_in[:, ib * KW + t5 * 128: ib * KW + t5 * 128 + rows],
                                     Wv[:, ib * E + c0: ib * E + c0 + w],
                                     start=(ib == 0),
                                     stop=(ib == IB - 1 and not has_bias))
                if has_bias:
                    nc.tensor.matmul(ps[:rows, :w], onesr[:, :rows],
                                     browp[:, E + c0:E + c0 + w],
                                     start=False, stop=True)
                # scatter the 64-col head blocks into stride-65 slots, *wsum
                src = ps[:rows, :w].rearrange("p (h c) -> p h c", c=64)
                dst = vp[:rows].rearrange("p (h c) -> p h c", c=65)[:, c0 // 64:(c0 + w) // 64, 0:64]
                nc.scalar.activation(dst, src, AF.Copy, scale=dram["_wsum"])
            nc.vector.memset(vp[:rows, 64::65], 1.0)
            if rows < 128:
                # zero the unwritten rows: they are contracted (against the
                # zero rows of E1t) in the last tile's chunk1 matmul
                nc.gpsimd.memset(vp[rows:64, :], 0.0)
                nc.gpsimd.memset(vp[64:128, :], 0.0)
            vplus.append(vp)

        # zero-padded per-tile E1 tiles (rows 32:128 stay zero so the ctx
        # chunk1 matmul can contract over the full 128 partitions)
        E1t = []
        for tt in range(NT):
            t = cp.tile([128, H * 128], bf16, tag=f"E1t{tt}")
            nc.gpsimd.memset(t[32:64, :], 0.0)
            nc.gpsimd.memset(t[64:128, :], 0.0)
            E1t.append(t)

        # per-tile LayerNorm stats, finalized after the loop
        s2x4 = wp.tile([128, NT], f32, tag="s2x4", bufs=1)
        sq4 = wp.tile([128, NT], f32, tag="sq4", bufs=1)
        mean4 = wp.tile([128, NT], f32, tag="mean4", bufs=1)
        var4 = wp.tile([128, NT], f32, tag="var4", bufs=1)
        m24 = wp.tile([128, NT], f32, tag="m24", bufs=1)

        # ---------------- stage B: banded attention, transposed scores ------
        for tt in range(NT):
            # scores^T: k on partitions.  chunk0 = k rows 0..127 of the 160
            # window, chunk1 = k rows 128..159 (32 partitions, packed 4/psum).
            sc0 = [pS.tile([128, 512], f32, tag="S", name=f"sc0_{tt}_{g}")
                   for g in range(3)]
            sc1 = [pS1.tile([32, 512], f32, tag="S1", name=f"sc1_{tt}_{g}")
                   for g in range(3)]
            for h in range(H):
                g, j = h // 4, h % 4
                ob = h // 2
                qm = (qTe if h % 2 == 0 else qTo)[tt]
                qs = qm[:, (h // 2) * 128:(h // 2) * 128 + 128]
                nc.tensor.matmul(sc0[g][:, j * 128:(j + 1) * 128],
                                 kT_sb[ob][:, tt * 128: tt * 128 + 128],
                                 qs, start=True, stop=True)
                nc.tensor.matmul(sc1[g][0:32, j * 128:(j + 1) * 128],
                                 kT_sb[ob][:, tt * 128 + 128: tt * 128 + 160],
                                 qs, start=True, stop=True)

            # V chains for the windows this tile's ctx needs, emitted here so
            # the PE chews scores first while V inputs finish streaming
            while len(vplus) < tt + 2:
                emit_vchain(len(vplus))

            # mask (band * am, band carries 1/sqrt(d)) then exp -> E tiles bf16
            E0 = wp.tile([128, H * 128], bf16, tag="E0", bufs=2)
            E1 = E1t[tt]
            for g in range(3):
                nc.vector.scalar_tensor_tensor(E0[:, g * 512:(g + 1) * 512],
                                               sc0[g][:], amc0[:, tt:tt + 1], band0[:],
                                               op0=ALU.mult, op1=ALU.mult)
                nc.vector.scalar_tensor_tensor(E1[0:32, g * 512:(g + 1) * 512],
                                               sc1[g][0:32, :], amc1[:, tt:tt + 1], band1[:],
                                               op0=ALU.mult, op1=ALU.mult)
            # split per head-group: ctx for heads 0-5 starts after the
            # first halves while the second halves are still exp'ing
            nc.scalar.activation(E0[:, 0:768], E0[:, 0:768], AF.Exp)
            nc.scalar.activation(E1[0:32, 0:768], E1[0:32, 0:768], AF.Exp)
            nc.scalar.activation(E0[:, 768:1536], E0[:, 768:1536], AF.Exp)
            nc.scalar.activation(E1[0:32, 768:1536], E1[0:32, 768:1536], AF.Exp)

            # context + Z in one PSUM tile per 6 heads: cols j*65..j*65+63 are
            # ctx (wsum-scaled v), col j*65+64 accumulates sum_tile(E); the
            # final K=1 matmul adds (wsum*(V_sum - V_tilesum) | S-160).
            # the K=1 vdz matmul goes FIRST with start=True: on HW, start
            # marks the whole 2KB zero-region pending, so it must be the only
            # start in this bank; every head matmul then accumulates.
            Zw = wp.tile([128, H], f32, tag="Zw", bufs=2)
            cxs = []
            for g2 in range(2):
                cx = pS.tile([128, 512], f32, tag="S")
                nc.tensor.matmul(cx[:, 0:390], onesr[:],
                                 vdzsb[:, tt * 780 + g2 * 390: tt * 780 + (g2 + 1) * 390],
                                 start=True, stop=False)
                for j in range(6):
                    h = g2 * 6 + j
                    nc.tensor.matmul(cx[:, j * 65: j * 65 + 65],
                                     E0[:, h * 128:(h + 1) * 128],
                                     vplus[tt][:, h * 65: h * 65 + 65],
                                     start=False, stop=False)
                    nc.tensor.matmul(cx[:, j * 65: j * 65 + 65],
                                     E1[:, h * 128:(h + 1) * 128],
                                     vplus[tt + 1][:, h * 65: h * 65 + 65],
                                     start=False, stop=(j == 5))
                nc.vector.tensor_copy(Zw[:, g2 * 6:(g2 + 1) * 6], cx[:, 64:390:65])
                cxs.append(cx)

            Zr = wp.tile([128, H], f32, tag="Zr", bufs=2)
            nc.vector.reciprocal(Zr[:], Zw[:])
            tmp = wp.tile([128, 384], f32, tag="ctmp", bufs=2)
            for g2 in range(2):
                # (ctx * 1/Z) + q over all 6 heads at once: cx read through a
                # stride-65 view, Zr through a stride-0 free-dim broadcast
                cxv = cxs[g2][:, 0:390].rearrange("p (h c) -> p h c", c=65)[:, :, 0:64]
                zrv = Zr[:, g2 * 6:(g2 + 1) * 6].unsqueeze(2).broadcast_to([128, 6, 64])
                tv = tmp[:].rearrange("p (h c) -> p h c", c=64)
                nc.vector.tensor_mul(tv, cxv, zrv)
                qv = q_tok[tt][:, g2 * 384:(g2 + 1) * 384]
                nc.vector.tensor_add(qv, tmp[:], qv)

            # LayerNorm stats for this tile on the DVE (the ACT engine's FIFO
            # would otherwise serialize the next tile's exp behind them, and
            # its Sqrt table-load thrashes against the Exp table)
            junk = wp.tile([128, E], bf16, tag="junk", bufs=2)
            nc.scalar.activation(junk[:], q_tok[tt][:], AF.Copy,
                                 accum_out=s2x4[:, tt:tt + 1])
            nc.scalar.activation(junk[:], q_tok[tt][:], AF.Square,
                                 accum_out=sq4[:, tt:tt + 1])
            nc.vector.tensor_scalar_mul(mean4[:, tt:tt + 1], s2x4[:, tt:tt + 1],
                                        1.0 / E)
            nc.vector.tensor_scalar_mul(var4[:, tt:tt + 1], sq4[:, tt:tt + 1],
                                        1.0 / E)
            nc.vector.tensor_mul(m24[:, tt:tt + 1], mean4[:, tt:tt + 1],
                                 mean4[:, tt:tt + 1])
            nc.vector.tensor_sub(var4[:, tt:tt + 1], var4[:, tt:tt + 1],
                                 m24[:, tt:tt + 1])

        # ---------------- LayerNorm tail ------------------------------------
        # Sqrts are emitted after the last Exp in the ACT FIFO (one table
        # load) but per-column, so tiles 0-2 finalize and DMA out while tile
        # 3's attention is still running.
        sd4 = wp.tile([128, NT], f32, tag="sd4", bufs=1)
        rstd4 = wp.tile([128, NT], f32, tag="rstd4", bufs=1)
        for tt in range(NT):
            nc.scalar.activation(sd4[:, tt:tt + 1], var4[:, tt:tt + 1],
                                 AF.Sqrt, bias=epst[:])
            nc.vector.reciprocal(rstd4[:, tt:tt + 1], sd4[:, tt:tt + 1])
            u = wp.tile([128, E], f32, tag="u", bufs=2)
            rsb = rstd4[:, tt:tt + 1].broadcast_to([128, E])
            nc.vector.scalar_tensor_tensor(u[:], q_tok[tt][:], mean4[:, tt:tt + 1],
                                           rsb, op0=ALU.subtract, op1=ALU.mult)
            if not unit_gamma:
                nc.vector.tensor_mul(u[:], u[:], gb[:])
                nc.vector.tensor_add(u[:], u[:], bb[:])
            sync.dma_start(dram["out"][tt * 128:(tt + 1) * 128, :], u[:])


def _build(has_bias, unit_gamma, wsum):
    key = (has_bias, unit_gamma, float(wsum))
    if key in _cache:
        return _cache[key]
    nc = bacc.Bacc("TRN2", target_bir_lowering=False, debug=False,
                   num_devices=N_CORES)
    dram = {}

    def din(name, shape, dt):
        dram[name] = nc.dram_tensor(name, list(shape), dt, kind="ExternalInput").ap()

    din("qT", (128, IB * R), bf16)
    din("kT", (128, IB * KW), bf16)
    din("vT", (128, IB * KW), bf16)
    din("WqT", (128, IB * E), bf16)
    din("WkT", (128, IB * E), bf16)
    din("WvT", (128, IB * E), bf16)
    din("band0", (128, 512), bf16)
    din("band1", (32, 512), bf16)
    din("amcol0", (128, NT), f32)
    din("amcol1", (32, NT), f32)
    din("vdz", (1, NT * 780), bf16)
    din("bk_col", (128, IB), f32)
    if has_bias:
        din("brow", (1, 2 * E), bf16)
    if not unit_gamma:
        din("grow", (1, 2 * E), bf16)
    dram["out"] = nc.dram_tensor("out", [R, E], f32, kind="ExternalOutput").ap()
    dram["_wsum"] = float(wsum)

    with tile.TileContext(nc) as tc:
        _emit(nc, tc, dram, has_bias, unit_gamma)
    nc.compile()
    _cache[key] = nc
    return nc


def _pack_feature_major(x_te, cols):
    # [tokens, 768] -> [128, 6*cols] with block ib at cols [ib*cols,(ib+1)*cols)
    t = x_te.shape[0]
    out = np.zeros((128, IB * cols), dtype=bf)
    xT = np.ascontiguousarray(x_te.T)  # [768, tokens]
    for ib in range(IB):
        out[:, ib * cols: ib * cols + t] = xT[ib * 128:(ib + 1) * 128, :]
    return out


def prepare_in_maps(**inputs):
    query = np.asarray(inputs["query"], np.float32)
    key = np.asarray(inputs["key"], np.float32)
    value = np.asarray(inputs["value"], np.float32)
    am = np.asarray(inputs["attention_mask"], np.float32)
    Wq = np.asarray(inputs["Wq"], np.float32)
    bq = np.asarray(inputs["bq"], np.float32)
    Wk = np.asarray(inputs["Wk"], np.float32)
    bk = np.asarray(inputs["bk"], np.float32)
    Wv = np.asarray(inputs["Wv"], np.float32)
    bv = np.asarray(inputs["bv"], np.float32)
    ww = np.asarray(inputs["window_weights"], np.float32)
    gamma = np.asarray(inputs["gamma"], np.float32)
    beta = np.asarray(inputs["beta"], np.float32)

    has_bias = bool(np.any(bq) or np.any(bv))
    unit_gamma = bool(np.all(gamma == 1.0) and np.all(beta == 0.0))
    wsum = float(ww.sum())
    inv_sqrt_d = 1.0 / np.sqrt(D)

    def wpack(W):
        # W.T rows [ib*128, (ib+1)*128) as [128, 768] blocks side by side
        WT = np.ascontiguousarray(W.T)
        out = np.empty((128, IB * E), dtype=bf)
        for ib in range(IB):
            out[:, ib * E:(ib + 1) * E] = WT[ib * 128:(ib + 1) * 128, :]
        return out

    WqP, WkP, WvP = wpack(Wq), wpack(Wk), wpack(Wv)

    # band masks in tile-local coords (identical for every tile / core):
    # chunk0: k_idx p in [0,128): valid iff 0 <= p - q <= 32
    # chunk1: k_idx 128+r:        valid iff 0 <= 128 + r - q <= 32
    p = np.arange(128)[:, None]
    q = np.arange(128)[None, :]
    b0 = ((p - q >= 0) & (p - q <= 2 * HALO)).astype(np.float32) * inv_sqrt_d
    r = np.arange(32)[:, None]
    b1 = ((128 + r - q >= 0) & (128 + r - q <= 2 * HALO)).astype(np.float32) * inv_sqrt_d
    band0 = np.ascontiguousarray(np.tile(b0, (1, 4))).astype(bf)
    band1 = np.ascontiguousarray(np.tile(b1, (1, 4))).astype(bf)

    brow = np.concatenate([bq, bv])[None, :].astype(bf)
    grow = np.concatenate([gamma, beta])[None, :].astype(bf)
    bk_col = np.ascontiguousarray(bk.reshape(IB, 128).T).astype(np.float32)

    in_maps = []
    for c in range(N_CORES):
        b, r0 = c // 2, (c % 2) * R
        lo, hi = r0 - HALO, r0 + R + HALO
        s_lo, s_hi = max(lo, 0), min(hi, S)

        kwin = np.zeros((KW, E), np.float32)
        kwin[s_lo - lo:s_hi - lo] = key[b, s_lo:s_hi]
        vwin = np.zeros((KW, E), np.float32)
        vwin[s_lo - lo:s_hi - lo] = value[b, s_lo:s_hi]

        # attention-mask per k-partition for each tile (value at padded rows
        # is irrelevant -- the score there is already 0)
        kg = lo + np.arange(NT)[None, :] * 128 + np.arange(TW)[:, None]  # [TW, NT]
        amv = am[b][np.clip(kg, 0, S - 1)].astype(np.float32)
        amcol0 = np.ascontiguousarray(amv[:128])
        amcol1 = np.ascontiguousarray(amv[128:])

        # vdz row: per tile, per head: wsum*(V_sum - V_tilesum) | (S - 160)
        vsum = value[b].sum(axis=0) @ Wv.T + S * bv          # [768]
        vdzc = np.empty((NT, H * 65), np.float32)
        for tt in range(NT):
            w_lo, w_hi = max(lo + tt * 128, 0), min(lo + tt * 128 + TW, S)
            vts = value[b, w_lo:w_hi].sum(axis=0) @ Wv.T + (w_hi - w_lo) * bv
            vd = (wsum * (vsum - vts)).reshape(H, D)
            blk = vdzc[tt].reshape(H, 65)
            blk[:, :64] = vd
            blk[:, 64] = float(S - TW)
        vdz = vdzc.reshape(1, NT * 780).astype(bf)

        in_maps.append({
            "qT": _pack_feature_major(query[b, r0:r0 + R], R),
            "kT": _pack_feature_major(kwin, KW),
            "vT": _pack_feature_major(vwin, KW),
            "WqT": WqP, "WkT": WkP, "WvT": WvP,
            "band0": band0, "band1": band1,
            "amcol0": amcol0, "amcol1": amcol1,
            "vdz": vdz, "bk_col": bk_col,
            **({"brow": brow} if has_bias else {}),
            **({} if unit_gamma else {"grow": grow}),
        })

    return in_maps, has_bias, unit_gamma, wsum


def gather(results):
    out = np.empty((B, S, E), np.float32)
    for c in range(N_CORES):
        b, r0 = c // 2, (c % 2) * R
        out[b, r0:r0 + R] = results[c]["out"]
    return out


def kernel(**inputs):
    in_maps, has_bias, unit_gamma, wsum = prepare_in_maps(**inputs)
    nc = _build(has_bias, unit_gamma, wsum)
    res = run_bass_kernel_spmd(nc, in_maps, core_ids=list(range(N_CORES)))
    return gather(res.results)
